# revision 27
# baseline (speedup 1.0000x reference)
"""CTLSTMCell fused kernel for Trainium2, 8 NeuronCores.

Sharding: tensor-parallel over the D=1024 feature columns. Core c owns
columns [c*128, (c+1)*128) and computes all 7 gate blocks for that slice:
    gates[:, g*1024 + c*128 : g*1024 + (c+1)*128]  for g in 0..6
Each core runs the full batch (B=4096); the weight is split 8 ways and
stays resident in SBUF.

On-chip layout is [features, batch] (transposed): the contraction dim K of
the matmul must sit on SBUF partitions for both operands, W is naturally
K-major, and x is transposed once on the host. This also puts the bias on
partitions, so it fuses into the ScalarE activation op (func(scale*in+bias)).

Mixed precision: the error-tolerant gates (ig, ibg, dg) use fp8 E4M3 with
MatmulPerfMode.DoubleRow end to end: operands are [K=128, 2, N] APs
contracting 256 per call (2 fp8 weights per PE: same 1 cyc/row occupancy
as bf16 but half the calls). The error-sensitive gates (fg, og, zg, fbg —
they feed sigmoid*state products or the output directly) use bf16 matmuls
for K 0..1535 plus 2 DoubleRow fp8 calls for K 1536..2047, accumulated in
the same PSUM bank (saves 8 of 88 matmul slots per tile; measured worst
rel err 1.42e-2 vs the 2e-2 gate — full-fp8 on these gates fails at
2.9e-2). All matmul operands are pre-scaled on the host (x*4, W*64, bf16
included so both precisions share one PSUM scale); the 1/256 rides the
ScalarE activation's scale input. cell/cellbar/decay outputs store as
bf16 (+0.4% rel, inside budget) and are upcast on the host; output_gate
(the tightest output) stores f32.

softplus(SCALE*g)/SCALE ≈ ln2/SCALE + g/2 + SCALE*g²/8 (|SCALE*g| ≤ ~0.35,
trunc err ~6e-5 abs vs 0.17 abs tol) via Square+Identity ACTs keeps the
whole epilogue inside the sigmoid_and_others table set: no ~2.7µs ACT
table switches.

DMA batching: the Sync engine needs ~0.7-1.1µs per dma_start (each one
fans out over 16 hardware queues, ~330 GB/s aggregate), so per n-tile the
12 bf16 k-chunks land in ONE [128,12,512] DMA (chunk index on the free
dim), the 8 DoubleRow calls in ONE [128,8,2,512] DMA, and both cell
states in ONE [128,2,512] DMA; matmul operands integer-index the chunk
(xt[:, kc, :]). Tile 0 streams W/x in half pieces (finer pieces starve —
startup is DMA-bandwidth-bound) and runs its full-fp8 DoubleRow calls at
kc>=4 so w8/x80 are off the startup critical path.
"""

import numpy as np

D = 1024
B = 4096
K = 2 * D            # 2048 contraction
NCORES = 8
DLOC = D // NCORES   # 128 columns of D per core
KCH = 12             # bf16 k-chunks for sensitive gates (K 0..1535)
NC8T = 2             # fp8 DoubleRow tail calls for sensitive gates (K 1536..2047)
NC8 = K // 256       # 8 double-row calls (fp8)
NT = B // 512        # 8 batch tiles of 512
SCALE = 0.1          # softplus beta

XS = 4.0             # fp8 input scale
WS = 64.0            # fp8 weight scale
PS = XS * WS         # psum scale for fp8 gates

F8G = (0, 4, 6)      # ig, ibg, dg -> fp8 DoubleRow
BFG = (1, 2, 3, 5)   # fg, og, zg, fbg -> bf16
F8SLOT = {g: i for i, g in enumerate(F8G)}
BFSLOT = {g: i for i, g in enumerate(BFG)}
GC8 = len(F8G) * DLOC   # 384 fp8 gate columns per core
GCB = len(BFG) * DLOC   # 512 bf16 gate columns per core

_BUILT = {}


def _build():
    import concourse.bacc as bacc
    import concourse.mybir as mybir
    from concourse.tile import TileContext

    bf16 = mybir.dt.bfloat16
    fp8 = mybir.dt.float8e4
    f32 = mybir.dt.float32
    AF = mybir.ActivationFunctionType
    DR = mybir.MatmulPerfMode.DoubleRow

    nc = bacc.Bacc("TRN2")
    xT = nc.declare_dram_parameter("xT", [128, KCH, B], bf16, isOutput=False)
    x8T = nc.declare_dram_parameter("x8T", [128, NC8, 2, B], fp8, isOutput=False)
    Wc = nc.declare_dram_parameter("Wc", [128, KCH, GCB], bf16, isOutput=False)
    W8c = nc.declare_dram_parameter("W8c", [128, NC8, 2, GC8], fp8, isOutput=False)
    W8s = nc.declare_dram_parameter("W8s", [128, NC8T, 2, GCB], fp8, isOutput=False)
    bc = nc.declare_dram_parameter("bc", [DLOC, 8], f32, isOutput=False)
    cellsT = nc.declare_dram_parameter("cellsT", [DLOC, 2, B], f32, isOutput=False)
    coT = nc.declare_dram_parameter("coT", [DLOC, B], bf16, isOutput=True)
    cboT = nc.declare_dram_parameter("cboT", [DLOC, B], bf16, isOutput=True)
    dgoT = nc.declare_dram_parameter("dgoT", [DLOC, B], bf16, isOutput=True)
    ogoT = nc.declare_dram_parameter("ogoT", [DLOC, B], f32, isOutput=True)

    # Epilogue gate order: og last (its sigmoid goes straight to DRAM,
    # shortening the kernel tail).
    GORDER = [6, 3, 0, 1, 4, 5, 2]

    # Tile-0 piece boundaries: half pieces — finer pieces starve the
    # stream (startup is DMA-bandwidth-bound, ~732ns/chunk supply vs 864ns
    # consumption leaves no catch-up margin).
    P0 = [(0, 6), (6, KCH)]

    with TileContext(nc) as tc:
        with (
            tc.tile_pool(name="wpool", bufs=1) as wp,
            tc.tile_pool(name="xpool", bufs=2) as xp,
            tc.tile_pool(name="gpool", bufs=2) as gp,
            tc.tile_pool(name="tpool", bufs=1) as tp,
            tc.tile_pool(name="opool", bufs=2) as op_,
            tc.tile_pool(name="pspool", bufs=8, space="PSUM") as pp,
        ):
            # Weights and x tile 0 stream in matched pieces so the opening
            # matmuls only wait for piece 0; fp8 operands follow (tile 0
            # runs its DoubleRow calls after all bf16 ones).
            wps = {}
            x0ps = {}

            def load_piece(lo, hi):
                w_ = wp.tile([128, hi - lo, GCB], bf16, name=f"w_{lo}")
                nc.sync.dma_start(out=w_[:, :, :], in_=Wc[:, lo:hi, :])
                wps[lo] = w_
                xq = xp.tile([128, hi - lo, 512], bf16, tag=f"x{lo}",
                             name=f"x_0_{lo}")
                nc.sync.dma_start(out=xq[:, :, :], in_=xT[:, lo:hi, 0:512])
                x0ps[lo] = xq

            load_piece(*P0[0])
            # fp8 operands next: tile 0 runs DoubleRow call c at kc = 4 + c,
            # so they're needed ~3.5us after the stream starts.
            w8t = wp.tile([128, NC8, 2, GC8], fp8, name="w8")
            nc.sync.dma_start(out=w8t[:, :, :, :], in_=W8c[:, :, :, :])
            x80 = xp.tile([128, NC8, 2, 512], fp8, tag="x8", name="x8_0")
            nc.sync.dma_start(out=x80[:, :, :, :], in_=x8T[:, :, :, 0:512])
            load_piece(*P0[1])
            w8st = wp.tile([128, NC8T, 2, GCB], fp8, name="w8s")
            nc.sync.dma_start(out=w8st[:, :, :, :], in_=W8s[:, :, :, :])

            bt = wp.tile([128, 8], f32)
            nc.sync.dma_start(out=bt[:, :], in_=bc[:, :])

            def wap(kc):
                for lo, hi in P0:
                    if lo <= kc < hi:
                        return wps[lo][:, kc - lo, :]

            def load_x(n):
                ns = slice(n * 512, (n + 1) * 512)
                xt = xp.tile([128, KCH, 512], bf16, tag="x0", name=f"x_{n}")
                nc.sync.dma_start(out=xt[:, :, :], in_=xT[:, :, ns])
                x8 = xp.tile([128, NC8, 2, 512], fp8, tag="x8", name=f"x8_{n}")
                nc.sync.dma_start(out=x8[:, :, :, :], in_=x8T[:, :, :, ns])

                def xap(kc):
                    return xt[:, kc, :]

                return xap, x8

            def xap0(kc):
                for lo, hi in P0:
                    if lo <= kc < hi:
                        return x0ps[lo][:, kc - lo, :]

            xnext = (xap0, x80)

            for n in range(NT):
                ns = slice(n * 512, (n + 1) * 512)
                xap, x8t_n = xnext

                if n + 1 < NT:
                    xnext = load_x(n + 1)

                cells = gp.tile([128, 2, 512], f32, tag="cells")
                nc.sync.dma_start(out=cells[:, :, :], in_=cellsT[:, :, ns])
                ct = cells[:, 0, :]
                cbt = cells[:, 1, :]

                def mm_bf16(pts, g, kc):
                    s = BFSLOT[g]
                    nc.tensor.matmul(
                        pts[g][:, :],
                        wap(kc)[:, s * 128:(s + 1) * 128],
                        xap(kc),
                        start=(kc == 0),
                        stop=False,
                    )

                def mm_fp8s(pts, g, cp):
                    s = BFSLOT[g]
                    nc.tensor.matmul(
                        pts[g][:, :],
                        w8st[:, cp, :, s * 128:(s + 1) * 128],
                        x8t_n[:, NC8 - NC8T + cp, :, :],
                        start=False,
                        stop=(cp == NC8T - 1),
                        perf_mode=DR,
                    )

                def mm_fp8(pts, g, c):
                    s = F8SLOT[g]
                    nc.tensor.matmul(
                        pts[g][:, :],
                        w8t[:, c, :, s * 128:(s + 1) * 128],
                        x8t_n[:, c, :, :],
                        start=(c == 0),
                        stop=(c == NC8 - 1),
                        perf_mode=DR,
                    )

                # k-chunk outer, gate inner: all 7 PSUM banks accumulate in
                # lockstep. fp8 call c rides after the bf16 matmuls of its
                # second chunk (kc = 2c+1); on tile 0 all fp8 calls run after
                # the bf16 ones so their DMAs are off the startup path. The
                # last n-tile runs gate-outer instead: each gate finishes as
                # early as possible so only og's ACT+store trail the final
                # matmul.
                pts = {
                    g: pp.tile([128, 512], f32, tag="pt", name=f"pt_{n}_{g}")
                    for g in GORDER
                }
                if n == 0:
                    # full-fp8 DR calls start at kc=4 so w8/x80 have ~3.5us
                    # to land after the first W/x piece.
                    for kc in range(KCH):
                        for g in BFG:
                            mm_bf16(pts, g, kc)
                        if kc >= 4:
                            for g in F8G:
                                mm_fp8(pts, g, kc - 4)
                    for cp in range(NC8T):
                        for g in BFG:
                            mm_fp8s(pts, g, cp)
                elif n < NT - 1:
                    for kc in range(KCH):
                        for g in BFG:
                            mm_bf16(pts, g, kc)
                        if kc % 2 == 1:
                            for g in F8G:
                                mm_fp8(pts, g, kc // 2)
                    for c in range(NC8 - NC8T, NC8):
                        for g in F8G:
                            mm_fp8(pts, g, c)
                        for g in BFG:
                            mm_fp8s(pts, g, c - (NC8 - NC8T))
                else:
                    for g in GORDER:
                        if g in F8SLOT:
                            for c in range(NC8):
                                mm_fp8(pts, g, c)
                        elif g != 2:
                            for kc in range(KCH):
                                mm_bf16(pts, g, kc)
                            for cp in range(NC8T):
                                mm_fp8s(pts, g, cp)
                        # og (g=2) runs in 256-wide halves at the epilogue so
                        # half 1's ACT+store overlaps half 2's matmuls

                sqt = tp.tile([128, 512], f32, tag="ept")
                nc.scalar.activation(
                    sqt[:, :], pts[6][:, :], AF.Square, bias=bt[:, 6:7],
                    scale=np.sqrt(SCALE / 8.0) / PS,
                )
                aft = gp.tile([128, 512], f32, tag="spt")
                nc.scalar.activation(
                    aft[:, :], pts[6][:, :], AF.Identity, bias=bt[:, 7:8],
                    scale=0.5 / PS,
                )
                dgt = op_.tile([128, 512], bf16, tag="dgt")
                nc.vector.tensor_add(dgt[:, :], sqt[:, :], aft[:, :])
                nc.sync.dma_start(out=dgoT[:, ns], in_=dgt[:, :])

                cin = gp.tile([128, 512], f32, tag="cin")
                nc.scalar.activation(cin[:, :], pts[3][:, :], AF.Tanh, bias=bt[:, 3:4], scale=1.0 / PS)
                s_ig = gp.tile([128, 512], f32, tag="s_ig")
                nc.scalar.activation(
                    s_ig[:, :], pts[0][:, :], AF.Sigmoid, bias=bt[:, 0:1], scale=1.0 / PS
                )
                s_fg = gp.tile([128, 512], f32, tag="s_fg")
                nc.scalar.activation(s_fg[:, :], pts[1][:, :], AF.Sigmoid, bias=bt[:, 1:2], scale=1.0 / PS)

                t1 = tp.tile([128, 512], f32, tag="t1")
                nc.vector.tensor_mul(t1[:, :], s_fg[:, :], ct)
                t2 = tp.tile([128, 512], f32, tag="t2")
                nc.vector.tensor_mul(t2[:, :], s_ig[:, :], cin[:, :])
                cot = op_.tile([128, 512], bf16, tag="cot")
                nc.vector.tensor_add(cot[:, :], t1[:, :], t2[:, :])
                nc.sync.dma_start(out=coT[:, ns], in_=cot[:, :])

                s_ibg = gp.tile([128, 512], f32, tag="s_ibg")
                nc.scalar.activation(
                    s_ibg[:, :], pts[4][:, :], AF.Sigmoid, bias=bt[:, 4:5], scale=1.0 / PS
                )
                s_fbg = gp.tile([128, 512], f32, tag="s_fbg")
                nc.scalar.activation(s_fbg[:, :], pts[5][:, :], AF.Sigmoid, bias=bt[:, 5:6], scale=1.0 / PS)

                t3 = tp.tile([128, 512], f32, tag="t3")
                nc.vector.tensor_mul(t3[:, :], s_fbg[:, :], cbt)
                t4 = tp.tile([128, 512], f32, tag="t4")
                nc.vector.tensor_mul(t4[:, :], s_ibg[:, :], cin[:, :])
                cbot = op_.tile([128, 512], bf16, tag="cbot")
                nc.vector.tensor_add(cbot[:, :], t3[:, :], t4[:, :])
                nc.sync.dma_start(out=cboT[:, ns], in_=cbot[:, :])

                ogt = op_.tile([128, 512], f32, tag="ogt")
                if n == NT - 1:
                    s2 = BFSLOT[2]
                    for h in range(2):
                        hs = slice(h * 256, (h + 1) * 256)
                        for kc in range(KCH):
                            nc.tensor.matmul(
                                pts[2][:, hs],
                                wap(kc)[:, s2 * 128:(s2 + 1) * 128],
                                xap(kc)[:, hs],
                                start=(kc == 0),
                                stop=False,
                            )
                        for cp in range(NC8T):
                            nc.tensor.matmul(
                                pts[2][:, hs],
                                w8st[:, cp, :, s2 * 128:(s2 + 1) * 128],
                                x8t_n[:, NC8 - NC8T + cp, :, hs],
                                start=False,
                                stop=(cp == NC8T - 1),
                                perf_mode=DR,
                            )
                        nc.scalar.activation(
                            ogt[:, hs], pts[2][:, hs], AF.Sigmoid,
                            bias=bt[:, 2:3], scale=1.0 / PS,
                        )
                        nc.sync.dma_start(
                            out=ogoT[:, n * 512 + h * 256:n * 512 + (h + 1) * 256],
                            in_=ogt[:, hs],
                        )
                else:
                    nc.scalar.activation(ogt[:, :], pts[2][:, :], AF.Sigmoid, bias=bt[:, 2:3], scale=1.0 / PS)
                    nc.sync.dma_start(out=ogoT[:, ns], in_=ogt[:, :])

    nc.compile()
    return nc


def get_nc():
    if "nc" not in _BUILT:
        _BUILT["nc"] = _build()
    return _BUILT["nc"]


def _chunked(a):
    """[nch*128, M] -> [128, nch, M]: row kc*128+k -> [k, kc]."""
    nch = a.shape[0] // 128
    m = a.shape[1]
    return np.ascontiguousarray(a.reshape(nch, 128, m).transpose(1, 0, 2))


def _pack_dr(a8):
    """[nc8*256, M] fp8 -> [128, nc8, 2, M] DoubleRow layout.

    K_local(c, k, i) = c*256 + i*128 + k maps to [k, c, i].
    """
    nc8 = a8.shape[0] // 256
    m = a8.shape[1]
    return np.ascontiguousarray(a8.reshape(nc8, 2, 128, m).transpose(2, 0, 1, 3))


def make_in_maps(event_type_emb_i, hidden_t__i_minus_1, cell_t__i_minus_1,
                 cell_bar_i_minus_1, W, b):
    import ml_dtypes

    emb = np.asarray(event_type_emb_i, dtype=np.float32)
    h = np.asarray(hidden_t__i_minus_1, dtype=np.float32)
    cell = np.asarray(cell_t__i_minus_1, dtype=np.float32)
    cellbar = np.asarray(cell_bar_i_minus_1, dtype=np.float32)
    W = np.asarray(W, dtype=np.float32)
    b = np.asarray(b, dtype=np.float32)

    x = np.concatenate([emb, h], axis=1)                    # [4096, 2048]
    xT = _chunked(np.asarray((x.T[:KCH * 128] * XS).astype(ml_dtypes.bfloat16)))  # [128,12,4096]
    x8T = _pack_dr((x.T * XS).astype(ml_dtypes.float8_e4m3))    # [128,8,2,4096]
    cellsT = np.stack([cell.T, cellbar.T], axis=1)  # [1024, 2, 4096]

    in_maps = []
    for c in range(NCORES):
        colsb = np.concatenate(
            [np.arange(g * D + c * DLOC, g * D + (c + 1) * DLOC) for g in BFG]
        )
        cols8 = np.concatenate(
            [np.arange(g * D + c * DLOC, g * D + (c + 1) * DLOC) for g in F8G]
        )
        cols_all = np.concatenate(
            [np.arange(g * D + c * DLOC, g * D + (c + 1) * DLOC) for g in range(7)]
        )
        Wcb = _chunked(np.asarray((W[:KCH * 128, colsb] * WS).astype(ml_dtypes.bfloat16)))
        W8sp = _pack_dr((W[KCH * 128:, colsb] * WS).astype(ml_dtypes.float8_e4m3))
        W8 = _pack_dr((W[:, cols8] * WS).astype(ml_dtypes.float8_e4m3))
        b7 = b[cols_all].reshape(7, DLOC).T        # [128, 7]
        bcc = np.empty((DLOC, 8), dtype=np.float32)
        bcc[:, :7] = b7
        bcc[:, 7] = 0.5 * b7[:, 6] + np.log(2.0) / SCALE
        bcc[:, 6] = np.sqrt(SCALE / 8.0) * b7[:, 6]
        in_maps.append({
            "xT": xT,
            "x8T": x8T,
            "Wc": Wcb,
            "W8c": W8,
            "W8s": W8sp,
            "bc": bcc,
            "cellsT": np.ascontiguousarray(cellsT[c * DLOC:(c + 1) * DLOC]),
        })
    return in_maps


def assemble(results):
    outs = []
    for name in ("coT", "cboT", "dgoT", "ogoT"):
        full = np.empty((B, D), dtype=np.float32)
        for c, r in enumerate(results):
            full[:, c * DLOC:(c + 1) * DLOC] = r[name].T.astype(np.float32)
        outs.append(full)
    return tuple(outs)


def kernel(**inputs):
    from concourse.bass_utils import run_bass_kernel_spmd

    nc = get_nc()
    in_maps = make_in_maps(**inputs)
    res = run_bass_kernel_spmd(nc, in_maps, list(range(NCORES)))
    return assemble(res.results)


# revision 28
# speedup vs baseline: 1.0491x; 1.0491x over previous
"""CTLSTMCell fused kernel for Trainium2, 8 NeuronCores.

Sharding: tensor-parallel over the D=1024 feature columns. Core c owns
columns [c*128, (c+1)*128) and computes all 7 gate blocks for that slice:
    gates[:, g*1024 + c*128 : g*1024 + (c+1)*128]  for g in 0..6
Each core runs the full batch (B=4096); the weight is split 8 ways and
stays resident in SBUF.

On-chip layout is [features, batch] (transposed): the contraction dim K of
the matmul must sit on SBUF partitions for both operands, W is naturally
K-major, and x is transposed once on the host. This also puts the bias on
partitions, so it fuses into the ScalarE activation op (func(scale*in+bias)).

Mixed precision: the error-tolerant gates (ig, ibg, dg) use fp8 E4M3 with
MatmulPerfMode.DoubleRow end to end: operands are [K=128, 2, N] APs
contracting 256 per call (2 fp8 weights per PE: same 1 cyc/row occupancy
as bf16 but half the calls). The error-sensitive gates (fg, og, zg, fbg —
they feed sigmoid*state products or the output directly) use bf16 matmuls
for K 0..1535 plus 2 DoubleRow fp8 calls for K 1536..2047, accumulated in
the same PSUM bank (saves 8 of 88 matmul slots per tile; measured worst
rel err 1.42e-2 vs the 2e-2 gate — full-fp8 on these gates fails at
2.9e-2). All matmul operands are pre-scaled on the host (x*4, W*64, bf16
included so both precisions share one PSUM scale); the 1/256 rides the
ScalarE activation's scale input. cell/cellbar/decay outputs store as
bf16 (+0.4% rel, inside budget) and are upcast on the host; output_gate
(the tightest output) stores f32.

softplus(SCALE*g)/SCALE ≈ ln2/SCALE + g/2 + SCALE*g²/8 (|SCALE*g| ≤ ~0.35,
trunc err ~6e-5 abs vs 0.17 abs tol) via Square+Identity ACTs keeps the
whole epilogue inside the sigmoid_and_others table set: no ~2.7µs ACT
table switches.

DMA batching: the Sync engine needs ~0.7-1.1µs per dma_start (each one
fans out over 16 hardware queues, ~330 GB/s aggregate), so per n-tile the
12 bf16 k-chunks land in ONE [128,12,512] DMA (chunk index on the free
dim), the 8 DoubleRow calls in ONE [128,8,2,512] DMA, and both cell
states in ONE [128,2,512] DMA; matmul operands integer-index the chunk
(xt[:, kc, :]). Tile 0 streams W/x in half pieces (finer pieces starve —
startup is DMA-bandwidth-bound) and runs its full-fp8 DoubleRow calls at
kc>=4 so w8/x80 are off the startup critical path.
"""

import numpy as np

D = 1024
B = 4096
K = 2 * D            # 2048 contraction
NCORES = 8
DLOC = D // NCORES   # 128 columns of D per core
KCH = 12             # bf16 k-chunks for sensitive gates (K 0..1535)
NC8T = 2             # fp8 DoubleRow tail calls for sensitive gates (K 1536..2047)
NC8 = K // 256       # 8 double-row calls (fp8)
NT = B // 512        # 8 batch tiles of 512
SCALE = 0.1          # softplus beta

XS = 4.0             # fp8 input scale
WS = 64.0            # fp8 weight scale
PS = XS * WS         # psum scale for fp8 gates

F8G = (0, 4, 6)      # ig, ibg, dg -> fp8 DoubleRow
BFG = (1, 2, 3, 5)   # fg, og, zg, fbg -> bf16
F8SLOT = {g: i for i, g in enumerate(F8G)}
BFSLOT = {g: i for i, g in enumerate(BFG)}
GC8 = len(F8G) * DLOC   # 384 fp8 gate columns per core
GCB = len(BFG) * DLOC   # 512 bf16 gate columns per core

_BUILT = {}


def _build():
    import concourse.bacc as bacc
    import concourse.mybir as mybir
    from concourse.tile import TileContext

    bf16 = mybir.dt.bfloat16
    fp8 = mybir.dt.float8e4
    f32 = mybir.dt.float32
    AF = mybir.ActivationFunctionType
    DR = mybir.MatmulPerfMode.DoubleRow

    nc = bacc.Bacc("TRN2")
    xT = nc.declare_dram_parameter("xT", [128, KCH, B], bf16, isOutput=False)
    x8T = nc.declare_dram_parameter("x8T", [128, NC8, 2, B], fp8, isOutput=False)
    Wc = nc.declare_dram_parameter("Wc", [128, KCH, GCB], bf16, isOutput=False)
    W8c = nc.declare_dram_parameter("W8c", [128, NC8, 2, GC8], fp8, isOutput=False)
    W8s = nc.declare_dram_parameter("W8s", [128, NC8T, 2, GCB], fp8, isOutput=False)
    bc = nc.declare_dram_parameter("bc", [DLOC, 8], f32, isOutput=False)
    cellsT = nc.declare_dram_parameter("cellsT", [DLOC, 2, B], f32, isOutput=False)
    coT = nc.declare_dram_parameter("coT", [DLOC, B], bf16, isOutput=True)
    cboT = nc.declare_dram_parameter("cboT", [DLOC, B], bf16, isOutput=True)
    dgoT = nc.declare_dram_parameter("dgoT", [DLOC, B], bf16, isOutput=True)
    ogoT = nc.declare_dram_parameter("ogoT", [DLOC, B], f32, isOutput=True)

    # Epilogue gate order: og last (its sigmoid goes straight to DRAM,
    # shortening the kernel tail).
    GORDER = [6, 3, 0, 1, 4, 5, 2]

    # Tile-0 piece boundaries: half pieces — finer pieces starve the
    # stream (startup is DMA-bandwidth-bound, ~732ns/chunk supply vs 864ns
    # consumption leaves no catch-up margin).
    P0 = [(0, 6), (6, KCH)]

    with TileContext(nc) as tc:
        with (
            tc.tile_pool(name="wpool", bufs=1) as wp,
            tc.tile_pool(name="xpool", bufs=2) as xp,
            tc.tile_pool(name="gpool", bufs=2) as gp,
            tc.tile_pool(name="tpool", bufs=1) as tp,
            tc.tile_pool(name="opool", bufs=2) as op_,
            tc.tile_pool(name="pspool", bufs=8, space="PSUM") as pp,
        ):
            # Weights and x tile 0 stream in matched pieces so the opening
            # matmuls only wait for piece 0; fp8 operands follow (tile 0
            # runs its DoubleRow calls after all bf16 ones).
            wps = {}
            x0ps = {}

            def load_piece(lo, hi):
                w_ = wp.tile([128, hi - lo, GCB], bf16, name=f"w_{lo}")
                nc.sync.dma_start(out=w_[:, :, :], in_=Wc[:, lo:hi, :])
                wps[lo] = w_
                xq = xp.tile([128, hi - lo, 512], bf16, tag=f"x{lo}",
                             name=f"x_0_{lo}")
                nc.sync.dma_start(out=xq[:, :, :], in_=xT[:, lo:hi, 0:512])
                x0ps[lo] = xq

            load_piece(*P0[0])
            # fp8 operands next: tile 0 runs DoubleRow call c at kc = 4 + c,
            # so they're needed ~3.5us after the stream starts.
            w8t = wp.tile([128, NC8, 2, GC8], fp8, name="w8")
            nc.sync.dma_start(out=w8t[:, :, :, :], in_=W8c[:, :, :, :])
            x80 = xp.tile([128, NC8, 2, 512], fp8, tag="x8", name="x8_0")
            nc.sync.dma_start(out=x80[:, :, :, :], in_=x8T[:, :, :, 0:512])
            load_piece(*P0[1])
            w8st = wp.tile([128, NC8T, 2, GCB], fp8, name="w8s")
            nc.sync.dma_start(out=w8st[:, :, :, :], in_=W8s[:, :, :, :])

            bt = wp.tile([128, 8], f32)
            nc.sync.dma_start(out=bt[:, :], in_=bc[:, :])

            def wap(kc):
                for lo, hi in P0:
                    if lo <= kc < hi:
                        return wps[lo][:, kc - lo, :]

            def load_x(n):
                ns = slice(n * 512, (n + 1) * 512)
                xt = xp.tile([128, KCH, 512], bf16, tag="x0", name=f"x_{n}")
                nc.sync.dma_start(out=xt[:, :, :], in_=xT[:, :, ns])
                x8 = xp.tile([128, NC8, 2, 512], fp8, tag="x8", name=f"x8_{n}")
                nc.sync.dma_start(out=x8[:, :, :, :], in_=x8T[:, :, :, ns])

                def xap(kc):
                    return xt[:, kc, :]

                return xap, x8

            def xap0(kc):
                for lo, hi in P0:
                    if lo <= kc < hi:
                        return x0ps[lo][:, kc - lo, :]

            xnext = (xap0, x80)

            for n in range(NT):
                ns = slice(n * 512, (n + 1) * 512)
                xap, x8t_n = xnext

                if n + 1 < NT:
                    xnext = load_x(n + 1)

                cells = gp.tile([128, 2, 512], f32, tag="cells")
                nc.sync.dma_start(out=cells[:, :, :], in_=cellsT[:, :, ns])
                ct = cells[:, 0, :]
                cbt = cells[:, 1, :]

                def mm_bf16(pts, g, kc):
                    s = BFSLOT[g]
                    nc.tensor.matmul(
                        pts[g][:, :],
                        wap(kc)[:, s * 128:(s + 1) * 128],
                        xap(kc),
                        start=(kc == 0),
                        stop=False,
                    )

                def mm_fp8s(pts, g, cp):
                    s = BFSLOT[g]
                    nc.tensor.matmul(
                        pts[g][:, :],
                        w8st[:, cp, :, s * 128:(s + 1) * 128],
                        x8t_n[:, NC8 - NC8T + cp, :, :],
                        start=False,
                        stop=(cp == NC8T - 1),
                        perf_mode=DR,
                    )

                def mm_fp8(pts, g, c):
                    s = F8SLOT[g]
                    nc.tensor.matmul(
                        pts[g][:, :],
                        w8t[:, c, :, s * 128:(s + 1) * 128],
                        x8t_n[:, c, :, :],
                        start=(c == 0),
                        stop=(c == NC8 - 1),
                        perf_mode=DR,
                    )

                # k-chunk outer, gate inner: all 7 PSUM banks accumulate in
                # lockstep. fp8 call c rides after the bf16 matmuls of its
                # second chunk (kc = 2c+1); on tile 0 all fp8 calls run after
                # the bf16 ones so their DMAs are off the startup path. The
                # last n-tile runs gate-outer instead: each gate finishes as
                # early as possible so only og's ACT+store trail the final
                # matmul.
                pts = {
                    g: pp.tile([128, 512], f32, tag="pt", name=f"pt_{n}_{g}")
                    for g in GORDER
                }
                if n == 0:
                    # full-fp8 DR calls start at kc=4 so w8/x80 have ~3.5us
                    # to land after the first W/x piece.
                    for kc in range(KCH):
                        for g in BFG:
                            mm_bf16(pts, g, kc)
                        if kc >= 4:
                            for g in F8G:
                                mm_fp8(pts, g, kc - 4)
                    for cp in range(NC8T):
                        for g in BFG:
                            mm_fp8s(pts, g, cp)
                elif n < NT - 1:
                    for kc in range(KCH):
                        for g in BFG:
                            mm_bf16(pts, g, kc)
                        if kc % 2 == 1:
                            for g in F8G:
                                mm_fp8(pts, g, kc // 2)
                    for c in range(NC8 - NC8T, NC8):
                        for g in F8G:
                            mm_fp8(pts, g, c)
                        for g in BFG:
                            mm_fp8s(pts, g, c - (NC8 - NC8T))
                else:
                    for g in GORDER:
                        if g in F8SLOT:
                            for c in range(NC8):
                                mm_fp8(pts, g, c)
                        else:
                            for kc in range(KCH):
                                mm_bf16(pts, g, kc)
                            for cp in range(NC8T):
                                mm_fp8s(pts, g, cp)

                sqt = tp.tile([128, 512], f32, tag="ept")
                nc.scalar.activation(
                    sqt[:, :], pts[6][:, :], AF.Square, bias=bt[:, 6:7],
                    scale=np.sqrt(SCALE / 8.0) / PS,
                )
                aft = gp.tile([128, 512], f32, tag="spt")
                nc.scalar.activation(
                    aft[:, :], pts[6][:, :], AF.Identity, bias=bt[:, 7:8],
                    scale=0.5 / PS,
                )
                dgt = op_.tile([128, 512], bf16, tag="dgt")
                nc.vector.tensor_add(dgt[:, :], sqt[:, :], aft[:, :])
                nc.sync.dma_start(out=dgoT[:, ns], in_=dgt[:, :])

                cin = gp.tile([128, 512], f32, tag="cin")
                nc.scalar.activation(cin[:, :], pts[3][:, :], AF.Tanh, bias=bt[:, 3:4], scale=1.0 / PS)
                s_ig = gp.tile([128, 512], f32, tag="s_ig")
                nc.scalar.activation(
                    s_ig[:, :], pts[0][:, :], AF.Sigmoid, bias=bt[:, 0:1], scale=1.0 / PS
                )
                s_fg = gp.tile([128, 512], f32, tag="s_fg")
                nc.scalar.activation(s_fg[:, :], pts[1][:, :], AF.Sigmoid, bias=bt[:, 1:2], scale=1.0 / PS)

                t1 = tp.tile([128, 512], f32, tag="t1")
                nc.vector.tensor_mul(t1[:, :], s_fg[:, :], ct)
                t2 = tp.tile([128, 512], f32, tag="t2")
                nc.vector.tensor_mul(t2[:, :], s_ig[:, :], cin[:, :])
                cot = op_.tile([128, 512], bf16, tag="cot")
                nc.vector.tensor_add(cot[:, :], t1[:, :], t2[:, :])
                nc.sync.dma_start(out=coT[:, ns], in_=cot[:, :])

                s_ibg = gp.tile([128, 512], f32, tag="s_ibg")
                nc.scalar.activation(
                    s_ibg[:, :], pts[4][:, :], AF.Sigmoid, bias=bt[:, 4:5], scale=1.0 / PS
                )
                s_fbg = gp.tile([128, 512], f32, tag="s_fbg")
                nc.scalar.activation(s_fbg[:, :], pts[5][:, :], AF.Sigmoid, bias=bt[:, 5:6], scale=1.0 / PS)

                t3 = tp.tile([128, 512], f32, tag="t3")
                nc.vector.tensor_mul(t3[:, :], s_fbg[:, :], cbt)
                t4 = tp.tile([128, 512], f32, tag="t4")
                nc.vector.tensor_mul(t4[:, :], s_ibg[:, :], cin[:, :])
                cbot = op_.tile([128, 512], bf16, tag="cbot")
                nc.vector.tensor_add(cbot[:, :], t3[:, :], t4[:, :])
                nc.sync.dma_start(out=cboT[:, ns], in_=cbot[:, :])

                ogt = op_.tile([128, 512], f32, tag="ogt")
                nc.scalar.activation(ogt[:, :], pts[2][:, :], AF.Sigmoid, bias=bt[:, 2:3], scale=1.0 / PS)
                nc.sync.dma_start(out=ogoT[:, ns], in_=ogt[:, :])

    nc.compile()
    return nc


def get_nc():
    if "nc" not in _BUILT:
        _BUILT["nc"] = _build()
    return _BUILT["nc"]


def _chunked(a):
    """[nch*128, M] -> [128, nch, M]: row kc*128+k -> [k, kc]."""
    nch = a.shape[0] // 128
    m = a.shape[1]
    return np.ascontiguousarray(a.reshape(nch, 128, m).transpose(1, 0, 2))


def _pack_dr(a8):
    """[nc8*256, M] fp8 -> [128, nc8, 2, M] DoubleRow layout.

    K_local(c, k, i) = c*256 + i*128 + k maps to [k, c, i].
    """
    nc8 = a8.shape[0] // 256
    m = a8.shape[1]
    return np.ascontiguousarray(a8.reshape(nc8, 2, 128, m).transpose(2, 0, 1, 3))


def make_in_maps(event_type_emb_i, hidden_t__i_minus_1, cell_t__i_minus_1,
                 cell_bar_i_minus_1, W, b):
    import ml_dtypes

    emb = np.asarray(event_type_emb_i, dtype=np.float32)
    h = np.asarray(hidden_t__i_minus_1, dtype=np.float32)
    cell = np.asarray(cell_t__i_minus_1, dtype=np.float32)
    cellbar = np.asarray(cell_bar_i_minus_1, dtype=np.float32)
    W = np.asarray(W, dtype=np.float32)
    b = np.asarray(b, dtype=np.float32)

    x = np.concatenate([emb, h], axis=1)                    # [4096, 2048]
    xT = _chunked(np.asarray((x.T[:KCH * 128] * XS).astype(ml_dtypes.bfloat16)))  # [128,12,4096]
    x8T = _pack_dr((x.T * XS).astype(ml_dtypes.float8_e4m3))    # [128,8,2,4096]
    cellsT = np.stack([cell.T, cellbar.T], axis=1)  # [1024, 2, 4096]

    in_maps = []
    for c in range(NCORES):
        colsb = np.concatenate(
            [np.arange(g * D + c * DLOC, g * D + (c + 1) * DLOC) for g in BFG]
        )
        cols8 = np.concatenate(
            [np.arange(g * D + c * DLOC, g * D + (c + 1) * DLOC) for g in F8G]
        )
        cols_all = np.concatenate(
            [np.arange(g * D + c * DLOC, g * D + (c + 1) * DLOC) for g in range(7)]
        )
        Wcb = _chunked(np.asarray((W[:KCH * 128, colsb] * WS).astype(ml_dtypes.bfloat16)))
        W8sp = _pack_dr((W[KCH * 128:, colsb] * WS).astype(ml_dtypes.float8_e4m3))
        W8 = _pack_dr((W[:, cols8] * WS).astype(ml_dtypes.float8_e4m3))
        b7 = b[cols_all].reshape(7, DLOC).T        # [128, 7]
        bcc = np.empty((DLOC, 8), dtype=np.float32)
        bcc[:, :7] = b7
        bcc[:, 7] = 0.5 * b7[:, 6] + np.log(2.0) / SCALE
        bcc[:, 6] = np.sqrt(SCALE / 8.0) * b7[:, 6]
        in_maps.append({
            "xT": xT,
            "x8T": x8T,
            "Wc": Wcb,
            "W8c": W8,
            "W8s": W8sp,
            "bc": bcc,
            "cellsT": np.ascontiguousarray(cellsT[c * DLOC:(c + 1) * DLOC]),
        })
    return in_maps


def assemble(results):
    outs = []
    for name in ("coT", "cboT", "dgoT", "ogoT"):
        full = np.empty((B, D), dtype=np.float32)
        for c, r in enumerate(results):
            full[:, c * DLOC:(c + 1) * DLOC] = r[name].T.astype(np.float32)
        outs.append(full)
    return tuple(outs)


def kernel(**inputs):
    from concourse.bass_utils import run_bass_kernel_spmd

    nc = get_nc()
    in_maps = make_in_maps(**inputs)
    res = run_bass_kernel_spmd(nc, in_maps, list(range(NCORES)))
    return assemble(res.results)
